# revision 11
# baseline (speedup 1.0000x reference)
"""Trainium2 Bass kernel for nn_ExpertRouter (MoE top-4 router).

Reference computation (see harness):
    logits = einsum('bsh,eh->bse', hidden, W) + bias        # [4,4096,32]
    probs  = softmax(logits, -1)
    topv, topi = top_k(probs, 4)
    dispatch = scatter(topv / topv.sum(-1, keepdims=True))  # dense [b,s,32]
    load = probs.mean((0,1)); loss = KL(uniform || load)/32 * 1e-4
    returns (dispatch_mask, loss, topi)

Sharding: data-parallel over batch*seq (16384 tokens -> 2048/core x 8).
Router weight + bias replicated. Each core emits its partial expert-load
vector; the (32-float) KL reduction happens host-side during unsharding.

The host shards x and re-lays it out for the device: per core it sends the
token-shard transposed (hidden-major) and split into an fp16 hi/lo pair
(x*16 = xh + xl to ~2^-22 relative), chunked to match SBUF tiles. W is
sent as W.T*4096 split the same way (scaling keeps both lo-planes out of
fp16-denormal range; exp() absorbs the 2^-16 factor via its scale arg).
The f32 product is recovered on the PE with 3 fp16 matmuls per chunk
(wh@xh + wh@xl + wl@xh; the dropped lo*lo term is ~1e-7 of the logits) at
1 cycle/row instead of fp32's 4, with 32-column weight loads.

Per-core device pipeline (4 groups of 512 tokens):
  plain DMA of xh/xl group planes [128, 16*512] fp16 (2 MiB each);
  all 16 wh@xh matmuls issue as soon as xh lands, then wh@xl + wl@xh
  -> logits' PSUM [32 experts, 512 tokens]
  -> copy+bias to SBUF (DVE tensor_scalar / ACT Identity, alternating)
  -> logitsT [128 tokens, 32 experts]: PE transpose (even tiles) or
     DVE 32x32 block transposes (odd tiles)
  -> ACT exp(scale=2^-16, accum_out = softmax denom)
  -> DVE max8/max_index = top-8 values+indices (covers top-4)
  -> dispatch mask = p * ((p >= v4) * 1/sum(top4))  (2 fused DVE ops)
  -> expert-load partial via tiny PE matmul p~^T @ (1/S), PSUM-accumulated.
"""

import numpy as np

# ---- problem constants (hardcoded per spec) ----
B, S, H = 4, 4096, 2048
E = 32          # experts
TOPK = 4
NCORES = 8
T = B * S                   # 16384 tokens
TPC = T // NCORES           # 2048 tokens per core
P = 128                     # partitions
NCHUNK = H // P             # 16 hidden chunks
GT = 512                    # tokens per group (one PSUM bank of logits)
NGROUP = TPC // GT          # 4 groups per core
NTILE = TPC // P            # 16 token tiles (128) per core

XSCALE = 16.0               # x pre-scale (host)
WSCALE = 4096.0             # W pre-scale (host)
EXP_SCALE = 1.0 / (XSCALE * WSCALE)

_CACHE = {}


def _build_nc():
    import concourse.bass as bass
    import concourse.tile as tile
    from concourse import bacc, mybir

    f32 = mybir.dt.float32
    f16 = mybir.dt.float16
    i32 = mybir.dt.int32
    u32 = mybir.dt.uint32
    AF = mybir.ActivationFunctionType
    Alu = mybir.AluOpType
    AX = mybir.AxisListType

    nc = bacc.Bacc("TRN2", target_bir_lowering=False, debug=False,
                   num_devices=NCORES)

    GW = NCHUNK * GT            # 8192 cols per group plane
    xh_d = nc.dram_tensor("xh", [NGROUP * P, GW], f16,
                          kind="ExternalInput").ap()
    xl_d = nc.dram_tensor("xl", [NGROUP * P, GW], f16,
                          kind="ExternalInput").ap()
    wh_d = nc.dram_tensor("wh", [P, NCHUNK * E], f16,
                          kind="ExternalInput").ap()
    wl_d = nc.dram_tensor("wl", [P, NCHUNK * E], f16,
                          kind="ExternalInput").ap()
    b_d = nc.dram_tensor("bias", [1, E], f16, kind="ExternalInput").ap()
    mask_d = nc.dram_tensor("mask_out", [P, NTILE * E], f32,
                            kind="ExternalOutput").ap()
    idx_d = nc.dram_tensor("idx_out", [P, NTILE * TOPK], i32,
                           kind="ExternalOutput").ap()
    load_d = nc.dram_tensor("load_out", [E, 1], f32,
                            kind="ExternalOutput").ap()

    with tile.TileContext(nc) as tc:
        with (
            tc.tile_pool(name="const", bufs=1) as constp,
            tc.tile_pool(name="xg", bufs=4) as xgpool,
            tc.tile_pool(name="lgps", bufs=3, space="PSUM") as lgpool,
            tc.tile_pool(name="ldps", bufs=1, space="PSUM") as ldpool,
            tc.tile_pool(name="tile", bufs=4) as tp,
            tc.tile_pool(name="small", bufs=4) as smallp,
            tc.tile_pool(name="acc", bufs=1) as accp,
        ):
            wh_sb = constp.tile([P, NCHUNK * E], f16, tag="wh")
            wl_sb = constp.tile([P, NCHUNK * E], f16, tag="wl")
            nc.sync.dma_start(wh_sb[:], wh_d[:])
            nc.sync.dma_start(wl_sb[:], wl_d[:])
            bias_sb = constp.tile([1, E], f16, tag="bias")
            nc.sync.dma_start(bias_sb[:], b_d[:])
            ones256 = constp.tile([1, GT], f16, tag="ones256")
            nc.gpsimd.memset(ones256[:], 256.0)

            mask_acc = accp.tile([P, NTILE * E], f32, tag="mask_acc")
            idx_acc = accp.tile([P, NTILE * 8], u32, tag="idx_acc")
            idx_c = accp.tile([P, NTILE * TOPK], u32, tag="idx_c")
            load_ps = ldpool.tile([E, 1], f32, tag="load")

            # hoist the exp ACT-table load into the DMA-wait prologue
            warm = constp.tile([1, 1], f32, tag="warm")
            nc.gpsimd.memset(warm[:], 0.0)
            nc.scalar.activation(warm[:], warm[:], AF.Exp)

            def emit_mm(g):
                """DMAs + the 48 fp16 matmuls + bias-copy for group g."""
                xh_t = xgpool.tile([P, GW], f16, tag="xh")
                nparts = 2
                step = GW // nparts
                for q in range(nparts):
                    nc.sync.dma_start(
                        xh_t[:, q * step:(q + 1) * step],
                        xh_d[g * P:(g + 1) * P, q * step:(q + 1) * step])
                xl_t = xgpool.tile([P, GW], f16, tag="xl")
                hw_ = GW // 2
                nc.sync.dma_start(xl_t[:, 0:hw_],
                                  xl_d[g * P:(g + 1) * P, 0:hw_])
                nc.sync.dma_start(xl_t[:, hw_:GW],
                                  xl_d[g * P:(g + 1) * P, hw_:GW])

                # hi terms first (only need the xh plane)
                lg = lgpool.tile([E, GT], f32, tag="lg")
                for j in range(NCHUNK):
                    nc.tensor.matmul(lg[:], wh_sb[:, j * E:(j + 1) * E],
                                     xh_t[:, j * GT:(j + 1) * GT],
                                     start=(j == 0), stop=False)
                for j in range(NCHUNK):
                    nc.tensor.matmul(lg[:], wl_sb[:, j * E:(j + 1) * E],
                                     xh_t[:, j * GT:(j + 1) * GT],
                                     start=False, stop=False)
                for j in range(NCHUNK):
                    nc.tensor.matmul(lg[:], wh_sb[:, j * E:(j + 1) * E],
                                     xl_t[:, j * GT:(j + 1) * GT],
                                     start=False, stop=False)

                # bias via K=1 matmul: (256*b)^T x (256*ones) = 65536*b
                nc.tensor.matmul(lg[:], bias_sb[:], ones256[:],
                                 start=False, stop=True)
                return lg

            def emit_post(g, lgs):
                """Per-128-token-tile softmax/top-k for group g."""
                for k in range(NGROUP):
                    i = NGROUP * g + k
                    # transpose [32, 128] -> [128, 32]: DVE 32x32 blocks
                    lgT = tp.tile([P, E], f32, tag="lgT_sb")
                    for blk in range(4):
                        nc.vector.transpose(
                            lgT[32 * blk:32 * blk + 32, 0:32],
                            lgs[0:32, 128 * k + 32 * blk:
                                128 * k + 32 * blk + 32])

                    # p~ = exp(true logits)
                    p_t = tp.tile([P, E], f32, tag="p")
                    nc.scalar.activation(p_t[:], lgT[:], AF.Exp,
                                         scale=EXP_SCALE)
                    # S = per-token softmax denom (DVE reduce)
                    s_t = smallp.tile([P, 1], f32, tag="s")
                    nc.vector.reduce_sum(s_t[:], p_t[:], axis=AX.X)
                    rs_t = smallp.tile([P, 1], f32, tag="rs")
                    nc.vector.reciprocal(rs_t[:], s_t[:])

                    # top-8 values (desc) + indices
                    mx8 = smallp.tile([P, 8], f32, tag="mx8")
                    nc.vector.max(mx8[:], p_t[:])
                    nc.vector.max_index(idx_acc[:, 8 * i:8 * i + 8],
                                        mx8[:], p_t[:])

                    t4 = smallp.tile([P, 1], f32, tag="t4")
                    nc.vector.reduce_sum(t4[:], mx8[:, 0:4], axis=AX.X)
                    r4 = smallp.tile([P, 1], f32, tag="r4")
                    nc.vector.reciprocal(r4[:], t4[:])

                    # mask = p~ * ((p~ >= v4) * r4)   (2 fused DVE ops)
                    ge = tp.tile([P, E], f32, tag="ge")
                    nc.vector.tensor_scalar(ge[:], p_t[:], mx8[:, 3:4],
                                            r4[:, 0:1], op0=Alu.is_ge,
                                            op1=Alu.mult)
                    nc.vector.tensor_mul(mask_acc[:, E * i:E * (i + 1)],
                                         p_t[:], ge[:])

                    # expert-load partial: load[e] += sum_t p~[t,e] / S[t]
                    nc.tensor.matmul(load_ps[:], p_t[:], rs_t[:],
                                     start=(i == 0), stop=(i == NTILE - 1))
                # compact this group's top-4 index columns (overlapped)
                nc.vector.tensor_copy(
                    idx_c[:, 4 * TOPK * g:4 * TOPK * (g + 1)]
                    .rearrange("p (k e) -> p k e", e=TOPK),
                    idx_acc[:, 32 * g:32 * (g + 1)]
                    .rearrange("p (k e) -> p k e", e=8)[:, :, 0:TOPK])
                # stream the mask out per finished group, overlapped
                if g < NGROUP - 1:
                    q = NTILE * E // NGROUP
                    nc.sync.dma_start(mask_d[:, g * q:(g + 1) * q],
                                      mask_acc[:, g * q:(g + 1) * q])

            # software-pipelined emission: postproc for group g is issued
            # after group g+1's matmuls so the in-order PE queue never
            # stalls on the (DVE/ACT) bias-copy of the current group
            prev = None
            for g in range(NGROUP):
                lgs = emit_mm(g)
                if prev is not None:
                    emit_post(prev[0], prev[1])
                prev = (g, lgs)
            emit_post(prev[0], prev[1])

            # ---- outputs ----
            q = NTILE * E // NGROUP
            nc.sync.dma_start(mask_d[:, (NGROUP - 1) * q:],
                              mask_acc[:, (NGROUP - 1) * q:])
            nc.sync.dma_start(idx_d[:], idx_c[:].bitcast(i32))
            ld_sb = smallp.tile([E, 1], f32, tag="ld_sb")
            nc.vector.tensor_copy(ld_sb[:], load_ps[:])
            nc.sync.dma_start(load_d[:], ld_sb[:])

    nc.finalize()
    return nc


def _get_nc():
    if "nc" not in _CACHE:
        _CACHE["nc"] = _build_nc()
    return _CACHE["nc"]


def _split16(a32):
    """fp16 hi/lo split: a32 ~= hi + lo with ~2^-22 relative error."""
    hi = a32.astype(np.float16)
    lo = (a32 - hi.astype(np.float32)).astype(np.float16)
    return hi, lo


def _prep_inputs(hidden_states, router_weight, expert_bias):
    x = np.asarray(hidden_states, np.float32).reshape(T, H)
    w = np.asarray(router_weight, np.float32)
    b = np.asarray(expert_bias, np.float32)

    # W' = 4096 * W^T, chunk-major fp16 hi/lo [128, NCHUNK*E]
    wt = (WSCALE * w.T).reshape(NCHUNK, P, E).transpose(1, 0, 2) \
        .reshape(P, NCHUNK * E)
    wh, wl = _split16(np.ascontiguousarray(wt))
    bias_row = np.ascontiguousarray(
        (256.0 * b).reshape(1, E).astype(np.float16))

    in_maps = []
    for c in range(NCORES):
        xs = XSCALE * x[c * TPC:(c + 1) * TPC].T        # [H, TPC] f32
        xh, xl = _split16(xs)
        # [H, TPC] -> (j 16, p 128, g 4, t 512) -> [g, p, j, t] flat
        def lay(a):
            return np.ascontiguousarray(
                a.reshape(NCHUNK, P, NGROUP, GT).transpose(2, 1, 0, 3)
            ).reshape(NGROUP * P, NCHUNK * GT)
        in_maps.append({
            "xh": lay(xh), "xl": lay(xl),
            "wh": wh, "wl": wl, "bias": bias_row,
        })
    return in_maps


def _postprocess(results):
    mask_shards = []
    idx_shards = []
    for c in range(NCORES):
        m = results[c]["mask_out"].reshape(P, NTILE, E)
        mask_shards.append(np.ascontiguousarray(m.transpose(1, 0, 2))
                           .reshape(TPC, E))
        ii = results[c]["idx_out"].reshape(P, NTILE, TOPK)
        idx_shards.append(np.ascontiguousarray(ii.transpose(1, 0, 2))
                          .reshape(TPC, TOPK))
    mask_full = np.concatenate(mask_shards, 0).reshape(B, S, E)
    idx_full = np.concatenate(idx_shards, 0).reshape(B, S, TOPK)

    # balance loss from per-core expert-load partials (tiny host reduction,
    # mirrors the reference fp32 arithmetic)
    load_sum = np.zeros((E,), np.float32)
    for c in range(NCORES):
        load_sum = load_sum + results[c]["load_out"].reshape(E).astype(
            np.float32)
    expert_load = load_sum / np.float32(T)
    tgt = np.float32(1.0 / E)
    balance = np.sum(tgt * (np.log(tgt) - np.log(expert_load)),
                     dtype=np.float32) / np.float32(E)
    loss = np.float32(balance * np.float32(1e-4))
    return (mask_full.astype(np.float32), loss,
            idx_full.astype(np.int32))


def run_on_device(in_maps, trace=False, **kw):
    from concourse.bass_utils import run_bass_kernel_spmd
    nc = _get_nc()
    return run_bass_kernel_spmd(nc, in_maps, list(range(NCORES)),
                                trace=trace, **kw)


def kernel(hidden_states, router_weight, expert_bias):
    in_maps = _prep_inputs(hidden_states, router_weight, expert_bias)
    res = run_on_device(in_maps)
    return _postprocess(res.results)


if __name__ == "__main__":
    rng = np.random.default_rng(0)
    hs = rng.standard_normal((B, S, H), dtype=np.float32)
    w = (0.02 * rng.standard_normal((E, H))).astype(np.float32)
    b = np.zeros((E,), np.float32)
    out = kernel(hs, w, b)
    print([getattr(o, "shape", o) for o in out])


# revision 12
# speedup vs baseline: 1.1016x; 1.1016x over previous
"""Trainium2 Bass kernel for nn_ExpertRouter (MoE top-4 router).

Reference computation (see harness):
    logits = einsum('bsh,eh->bse', hidden, W) + bias        # [4,4096,32]
    probs  = softmax(logits, -1)
    topv, topi = top_k(probs, 4)
    dispatch = scatter(topv / topv.sum(-1, keepdims=True))  # dense [b,s,32]
    load = probs.mean((0,1)); loss = KL(uniform || load)/32 * 1e-4
    returns (dispatch_mask, loss, topi)

Sharding: data-parallel over batch*seq (16384 tokens -> 2048/core x 8).
Router weight + bias replicated. Each core emits its partial expert-load
vector; the (32-float) KL reduction happens host-side during unsharding.

The host shards x and re-lays it out for the device: per core it sends the
token-shard transposed (hidden-major) and split into an fp16 hi/lo pair
(x*16 = xh + xl to ~2^-22 relative), chunked to match SBUF tiles. W is
sent as W.T*4096 split the same way (scaling keeps both lo-planes out of
fp16-denormal range; exp() absorbs the 2^-16 factor via its scale arg).
The f32 product is recovered on the PE with 3 fp16 matmuls per chunk
(wh@xh + wh@xl + wl@xh; the dropped lo*lo term is ~1e-7 of the logits) at
1 cycle/row instead of fp32's 4, with 32-column weight loads.

Per-core device pipeline (4 groups of 512 tokens):
  plain DMA of xh/xl group planes [128, 16*512] fp16 (2 MiB each);
  all 16 wh@xh matmuls issue as soon as xh lands, then wh@xl + wl@xh
  -> logits' PSUM [32 experts, 512 tokens]
  -> copy+bias to SBUF (DVE tensor_scalar / ACT Identity, alternating)
  -> logitsT [128 tokens, 32 experts]: PE transpose (even tiles) or
     DVE 32x32 block transposes (odd tiles)
  -> ACT exp(scale=2^-16, accum_out = softmax denom)
  -> DVE max8/max_index = top-8 values+indices (covers top-4)
  -> dispatch mask = p * ((p >= v4) * 1/sum(top4))  (2 fused DVE ops)
  -> expert-load partial via tiny PE matmul p~^T @ (1/S), PSUM-accumulated.
"""

import numpy as np

# ---- problem constants (hardcoded per spec) ----
B, S, H = 4, 4096, 2048
E = 32          # experts
TOPK = 4
NCORES = 8
T = B * S                   # 16384 tokens
TPC = T // NCORES           # 2048 tokens per core
P = 128                     # partitions
NCHUNK = H // P             # 16 hidden chunks
GT = 512                    # tokens per group (one PSUM bank of logits)
NGROUP = TPC // GT          # 4 groups per core
NTILE = TPC // P            # 16 token tiles (128) per core

XSCALE = 16.0               # x pre-scale (host)
WSCALE = 4096.0             # W pre-scale (host)
EXP_SCALE = 1.0 / (XSCALE * WSCALE)

_CACHE = {}


def _build_nc():
    import concourse.bass as bass
    import concourse.tile as tile
    from concourse import bacc, mybir

    f32 = mybir.dt.float32
    f16 = mybir.dt.float16
    i32 = mybir.dt.int32
    u32 = mybir.dt.uint32
    AF = mybir.ActivationFunctionType
    Alu = mybir.AluOpType
    AX = mybir.AxisListType

    nc = bacc.Bacc("TRN2", target_bir_lowering=False, debug=False,
                   num_devices=NCORES)

    GW = NCHUNK * GT            # 8192 cols per group plane
    xh_d = nc.dram_tensor("xh", [NGROUP * P, GW], f16,
                          kind="ExternalInput").ap()
    xl_d = nc.dram_tensor("xl", [NGROUP * P, GW], f16,
                          kind="ExternalInput").ap()
    wh_d = nc.dram_tensor("wh", [P, NCHUNK * E], f16,
                          kind="ExternalInput").ap()
    wl_d = nc.dram_tensor("wl", [P, NCHUNK * E], f16,
                          kind="ExternalInput").ap()
    b_d = nc.dram_tensor("bias", [E, 1], f32, kind="ExternalInput").ap()
    mask_d = nc.dram_tensor("mask_out", [P, NTILE * E], f32,
                            kind="ExternalOutput").ap()
    idx_d = nc.dram_tensor("idx_out", [P, NTILE * TOPK], i32,
                           kind="ExternalOutput").ap()
    load_d = nc.dram_tensor("load_out", [E, 1], f32,
                            kind="ExternalOutput").ap()

    with tile.TileContext(nc) as tc:
        with (
            tc.tile_pool(name="const", bufs=1) as constp,
            tc.tile_pool(name="xg", bufs=4) as xgpool,
            tc.tile_pool(name="lgps", bufs=3, space="PSUM") as lgpool,
            tc.tile_pool(name="ltps", bufs=4, space="PSUM") as ltpool,
            tc.tile_pool(name="ldps", bufs=1, space="PSUM") as ldpool,
            tc.tile_pool(name="lgsb", bufs=3) as lgsbp,
            tc.tile_pool(name="tile", bufs=4) as tp,
            tc.tile_pool(name="small", bufs=4) as smallp,
            tc.tile_pool(name="acc", bufs=1) as accp,
        ):
            wh_sb = constp.tile([P, NCHUNK * E], f16, tag="wh")
            wl_sb = constp.tile([P, NCHUNK * E], f16, tag="wl")
            nc.sync.dma_start(wh_sb[:], wh_d[:])
            nc.sync.dma_start(wl_sb[:], wl_d[:])
            bias_sb = constp.tile([E, 1], f32, tag="bias")
            nc.sync.dma_start(bias_sb[:], b_d[:])
            ident = constp.tile([E, E], f32, tag="ident")
            from concourse import masks
            masks.make_identity(nc, ident[:])

            mask_acc = accp.tile([P, NTILE * E], f32, tag="mask_acc")
            idx_acc = accp.tile([P, NTILE * 8], u32, tag="idx_acc")
            idx_c = accp.tile([P, NTILE * TOPK], u32, tag="idx_c")
            load_ps = ldpool.tile([E, 1], f32, tag="load")

            # hoist the exp ACT-table load into the DMA-wait prologue
            warm = constp.tile([1, 1], f32, tag="warm")
            nc.scalar.activation(warm[:], bias_sb[0:1, 0:1], AF.Exp)

            def emit_mm(g):
                """DMAs + the 48 fp16 matmuls + bias-copy for group g."""
                xh_t = xgpool.tile([P, GW], f16, tag="xh")
                nparts = 2
                step = GW // nparts
                for q in range(nparts):
                    nc.sync.dma_start(
                        xh_t[:, q * step:(q + 1) * step],
                        xh_d[g * P:(g + 1) * P, q * step:(q + 1) * step])
                xl_t = xgpool.tile([P, GW], f16, tag="xl")
                hw_ = GW // 2
                nc.sync.dma_start(xl_t[:, 0:hw_],
                                  xl_d[g * P:(g + 1) * P, 0:hw_])
                nc.sync.dma_start(xl_t[:, hw_:GW],
                                  xl_d[g * P:(g + 1) * P, hw_:GW])

                # hi terms first (only need the xh plane)
                lg = lgpool.tile([E, GT], f32, tag="lg")
                for j in range(NCHUNK):
                    nc.tensor.matmul(lg[:], wh_sb[:, j * E:(j + 1) * E],
                                     xh_t[:, j * GT:(j + 1) * GT],
                                     start=(j == 0), stop=False)
                for j in range(NCHUNK):
                    nc.tensor.matmul(lg[:], wl_sb[:, j * E:(j + 1) * E],
                                     xh_t[:, j * GT:(j + 1) * GT],
                                     start=False, stop=False)
                for j in range(NCHUNK):
                    nc.tensor.matmul(lg[:], wh_sb[:, j * E:(j + 1) * E],
                                     xl_t[:, j * GT:(j + 1) * GT],
                                     start=False, stop=(j == NCHUNK - 1))

                # PSUM -> SBUF with bias add (per-partition = per-expert)
                lgs = lgsbp.tile([E, GT], f32, tag="lgs")
                nc.scalar.activation(lgs[:], lg[:], AF.Identity,
                                     bias=bias_sb[:, 0:1], scale=1.0)
                return lgs

            def emit_post(g, lgs):
                """Per-128-token-tile softmax/top-k for group g."""
                for k in range(NGROUP):
                    i = NGROUP * g + k
                    # transpose [32, 128] -> [128, 32] on the PE
                    lgT = ltpool.tile([P, E], f32, tag="lgT_ps")
                    nc.tensor.transpose(
                        lgT[:], lgs[0:E, 128 * k:128 * (k + 1)], ident[:])

                    # p~ = exp(true logits)
                    p_t = tp.tile([P, E], f32, tag="p")
                    nc.scalar.activation(p_t[:], lgT[:], AF.Exp,
                                         scale=EXP_SCALE)
                    # S = per-token softmax denom (DVE reduce)
                    s_t = smallp.tile([P, 1], f32, tag="s")
                    nc.vector.reduce_sum(s_t[:], p_t[:], axis=AX.X)
                    rs_t = smallp.tile([P, 1], f32, tag="rs")
                    nc.vector.reciprocal(rs_t[:], s_t[:])

                    # top-8 values (desc) + indices
                    mx8 = smallp.tile([P, 8], f32, tag="mx8")
                    nc.vector.max(mx8[:], p_t[:])
                    nc.vector.max_index(idx_acc[:, 8 * i:8 * i + 8],
                                        mx8[:], p_t[:])

                    t4 = smallp.tile([P, 1], f32, tag="t4")
                    nc.vector.reduce_sum(t4[:], mx8[:, 0:4], axis=AX.X)
                    r4 = smallp.tile([P, 1], f32, tag="r4")
                    nc.vector.reciprocal(r4[:], t4[:])

                    # mask = p~ * ((p~ >= v4) * r4)   (2 fused DVE ops)
                    ge = tp.tile([P, E], f32, tag="ge")
                    nc.vector.tensor_scalar(ge[:], p_t[:], mx8[:, 3:4],
                                            r4[:, 0:1], op0=Alu.is_ge,
                                            op1=Alu.mult)
                    nc.vector.tensor_mul(mask_acc[:, E * i:E * (i + 1)],
                                         p_t[:], ge[:])

                    # expert-load partial: load[e] += sum_t p~[t,e] / S[t]
                    nc.tensor.matmul(load_ps[:], p_t[:], rs_t[:],
                                     start=(i == 0), stop=(i == NTILE - 1))
                # compact this group's top-4 index columns (overlapped)
                nc.vector.tensor_copy(
                    idx_c[:, 4 * TOPK * g:4 * TOPK * (g + 1)]
                    .rearrange("p (k e) -> p k e", e=TOPK),
                    idx_acc[:, 32 * g:32 * (g + 1)]
                    .rearrange("p (k e) -> p k e", e=8)[:, :, 0:TOPK])
                # stream the mask out per finished group, overlapped
                if g < NGROUP - 1:
                    q = NTILE * E // NGROUP
                    nc.sync.dma_start(mask_d[:, g * q:(g + 1) * q],
                                      mask_acc[:, g * q:(g + 1) * q])

            # software-pipelined emission: postproc for group g is issued
            # after group g+1's matmuls so the in-order PE queue never
            # stalls on the (DVE/ACT) bias-copy of the current group
            prev = None
            for g in range(NGROUP):
                lgs = emit_mm(g)
                if prev is not None:
                    emit_post(prev[0], prev[1])
                prev = (g, lgs)
            emit_post(prev[0], prev[1])

            # ---- outputs ----
            q = NTILE * E // NGROUP
            nc.sync.dma_start(mask_d[:, (NGROUP - 1) * q:],
                              mask_acc[:, (NGROUP - 1) * q:])
            nc.sync.dma_start(idx_d[:], idx_c[:].bitcast(i32))
            ld_sb = smallp.tile([E, 1], f32, tag="ld_sb")
            nc.vector.tensor_copy(ld_sb[:], load_ps[:])
            nc.sync.dma_start(load_d[:], ld_sb[:])

    nc.finalize()
    return nc


def _get_nc():
    if "nc" not in _CACHE:
        _CACHE["nc"] = _build_nc()
    return _CACHE["nc"]


def _split16(a32):
    """fp16 hi/lo split: a32 ~= hi + lo with ~2^-22 relative error."""
    hi = a32.astype(np.float16)
    lo = (a32 - hi.astype(np.float32)).astype(np.float16)
    return hi, lo


def _prep_inputs(hidden_states, router_weight, expert_bias):
    x = np.asarray(hidden_states, np.float32).reshape(T, H)
    w = np.asarray(router_weight, np.float32)
    b = np.asarray(expert_bias, np.float32)

    # W' = 4096 * W^T, chunk-major fp16 hi/lo [128, NCHUNK*E]
    wt = (WSCALE * w.T).reshape(NCHUNK, P, E).transpose(1, 0, 2) \
        .reshape(P, NCHUNK * E)
    wh, wl = _split16(np.ascontiguousarray(wt))
    bias_col = np.ascontiguousarray((XSCALE * WSCALE) * b.reshape(E, 1))

    in_maps = []
    for c in range(NCORES):
        xs = XSCALE * x[c * TPC:(c + 1) * TPC].T        # [H, TPC] f32
        xh, xl = _split16(xs)
        # [H, TPC] -> (j 16, p 128, g 4, t 512) -> [g, p, j, t] flat
        def lay(a):
            return np.ascontiguousarray(
                a.reshape(NCHUNK, P, NGROUP, GT).transpose(2, 1, 0, 3)
            ).reshape(NGROUP * P, NCHUNK * GT)
        in_maps.append({
            "xh": lay(xh), "xl": lay(xl),
            "wh": wh, "wl": wl, "bias": bias_col,
        })
    return in_maps


def _postprocess(results):
    mask_shards = []
    idx_shards = []
    for c in range(NCORES):
        m = results[c]["mask_out"].reshape(P, NTILE, E)
        mask_shards.append(np.ascontiguousarray(m.transpose(1, 0, 2))
                           .reshape(TPC, E))
        ii = results[c]["idx_out"].reshape(P, NTILE, TOPK)
        idx_shards.append(np.ascontiguousarray(ii.transpose(1, 0, 2))
                          .reshape(TPC, TOPK))
    mask_full = np.concatenate(mask_shards, 0).reshape(B, S, E)
    idx_full = np.concatenate(idx_shards, 0).reshape(B, S, TOPK)

    # balance loss from per-core expert-load partials (tiny host reduction,
    # mirrors the reference fp32 arithmetic)
    load_sum = np.zeros((E,), np.float32)
    for c in range(NCORES):
        load_sum = load_sum + results[c]["load_out"].reshape(E).astype(
            np.float32)
    expert_load = load_sum / np.float32(T)
    tgt = np.float32(1.0 / E)
    balance = np.sum(tgt * (np.log(tgt) - np.log(expert_load)),
                     dtype=np.float32) / np.float32(E)
    loss = np.float32(balance * np.float32(1e-4))
    return (mask_full.astype(np.float32), loss,
            idx_full.astype(np.int32))


def run_on_device(in_maps, trace=False, **kw):
    from concourse.bass_utils import run_bass_kernel_spmd
    nc = _get_nc()
    return run_bass_kernel_spmd(nc, in_maps, list(range(NCORES)),
                                trace=trace, **kw)


def kernel(hidden_states, router_weight, expert_bias):
    in_maps = _prep_inputs(hidden_states, router_weight, expert_bias)
    res = run_on_device(in_maps)
    return _postprocess(res.results)


if __name__ == "__main__":
    rng = np.random.default_rng(0)
    hs = rng.standard_normal((B, S, H), dtype=np.float32)
    w = (0.02 * rng.standard_normal((E, H))).astype(np.float32)
    b = np.zeros((E,), np.float32)
    out = kernel(hs, w, b)
    print([getattr(o, "shape", o) for o in out])


# revision 14
# speedup vs baseline: 1.1664x; 1.0588x over previous
"""Trainium2 Bass kernel for nn_ExpertRouter (MoE top-4 router).

Reference computation (see harness):
    logits = einsum('bsh,eh->bse', hidden, W) + bias        # [4,4096,32]
    probs  = softmax(logits, -1)
    topv, topi = top_k(probs, 4)
    dispatch = scatter(topv / topv.sum(-1, keepdims=True))  # dense [b,s,32]
    load = probs.mean((0,1)); loss = KL(uniform || load)/32 * 1e-4
    returns (dispatch_mask, loss, topi)

Sharding: data-parallel over batch*seq (16384 tokens -> 2048/core x 8).
Router weight + bias replicated. Each core emits its partial expert-load
vector; the (32-float) KL reduction happens host-side during unsharding.

The host shards x and re-lays it out for the device: per core it sends the
token-shard transposed (hidden-major) and split into an fp16 hi/lo pair
(x*16 = xh + xl to ~2^-22 relative), chunked to match SBUF tiles. W is
sent as W.T*4096 split the same way (scaling keeps both lo-planes out of
fp16-denormal range; exp() absorbs the 2^-16 factor via its scale arg).
The f32 product is recovered on the PE with 3 fp16 matmuls per chunk
(wh@xh + wh@xl + wl@xh; the dropped lo*lo term is ~1e-7 of the logits) at
1 cycle/row instead of fp32's 4, with 32-column weight loads.

Per-core device pipeline (4 groups of 512 tokens):
  plain DMA of xh/xl group planes [128, 16*512] fp16 (2 MiB each);
  all 16 wh@xh matmuls issue as soon as xh lands, then wh@xl + wl@xh
  -> logits' PSUM [32 experts, 512 tokens]
  -> copy+bias to SBUF (DVE tensor_scalar / ACT Identity, alternating)
  -> logitsT [128 tokens, 32 experts]: PE transpose (even tiles) or
     DVE 32x32 block transposes (odd tiles)
  -> ACT exp(scale=2^-16, accum_out = softmax denom)
  -> DVE max8/max_index = top-8 values+indices (covers top-4)
  -> dispatch mask = p * ((p >= v4) * 1/sum(top4))  (2 fused DVE ops)
  -> expert-load partial via tiny PE matmul p~^T @ (1/S), PSUM-accumulated.
"""

import numpy as np

# ---- problem constants (hardcoded per spec) ----
B, S, H = 4, 4096, 2048
E = 32          # experts
TOPK = 4
NCORES = 8
T = B * S                   # 16384 tokens
TPC = T // NCORES           # 2048 tokens per core
P = 128                     # partitions
NCHUNK = H // P             # 16 hidden chunks
GT = 512                    # tokens per group (one PSUM bank of logits)
NGROUP = TPC // GT          # 4 groups per core
NTILE = TPC // P            # 16 token tiles (128) per core

XSCALE = 16.0               # x pre-scale (host)
WSCALE = 4096.0             # W pre-scale (host)
EXP_SCALE = 1.0 / (XSCALE * WSCALE)

_CACHE = {}


def _build_nc():
    import concourse.bass as bass
    import concourse.tile as tile
    from concourse import bacc, mybir

    f32 = mybir.dt.float32
    f16 = mybir.dt.float16
    i32 = mybir.dt.int32
    u32 = mybir.dt.uint32
    AF = mybir.ActivationFunctionType
    Alu = mybir.AluOpType
    AX = mybir.AxisListType

    nc = bacc.Bacc("TRN2", target_bir_lowering=False, debug=False,
                   num_devices=NCORES)

    GW = NCHUNK * GT            # 8192 cols per group plane
    xh_d = nc.dram_tensor("xh", [NGROUP * P, GW], f16,
                          kind="ExternalInput").ap()
    xl_d = nc.dram_tensor("xl", [NGROUP * P, GW], f16,
                          kind="ExternalInput").ap()
    wh_d = nc.dram_tensor("wh", [P, NCHUNK * E], f16,
                          kind="ExternalInput").ap()
    wl_d = nc.dram_tensor("wl", [P, NCHUNK * E], f16,
                          kind="ExternalInput").ap()
    b_d = nc.dram_tensor("bias", [E, 1], f32, kind="ExternalInput").ap()
    mask_d = nc.dram_tensor("mask_out", [P, NTILE * E], f32,
                            kind="ExternalOutput").ap()
    idx_d = nc.dram_tensor("idx_out", [P, NTILE * TOPK], i32,
                           kind="ExternalOutput").ap()
    load_d = nc.dram_tensor("load_out", [E, 1], f32,
                            kind="ExternalOutput").ap()

    with tile.TileContext(nc) as tc:
        with (
            tc.tile_pool(name="const", bufs=1) as constp,
            tc.tile_pool(name="xg", bufs=4) as xgpool,
            tc.tile_pool(name="lgps", bufs=3, space="PSUM") as lgpool,
            tc.tile_pool(name="ltps", bufs=4, space="PSUM") as ltpool,
            tc.tile_pool(name="ldps", bufs=1, space="PSUM") as ldpool,
            tc.tile_pool(name="lgsb", bufs=3) as lgsbp,
            tc.tile_pool(name="tile", bufs=6) as tp,
            tc.tile_pool(name="small", bufs=6) as smallp,
            tc.tile_pool(name="acc", bufs=1) as accp,
        ):
            wh_sb = constp.tile([P, NCHUNK * E], f16, tag="wh")
            wl_sb = constp.tile([P, NCHUNK * E], f16, tag="wl")
            nc.sync.dma_start(wh_sb[:], wh_d[:])
            nc.sync.dma_start(wl_sb[:], wl_d[:])
            bias_sb = constp.tile([E, 1], f32, tag="bias")
            nc.sync.dma_start(bias_sb[:], b_d[:])
            ident = constp.tile([E, E], f32, tag="ident")
            from concourse import masks
            masks.make_identity(nc, ident[:])

            mask_acc = accp.tile([P, NTILE * E], f32, tag="mask_acc")
            idx_acc = accp.tile([P, NTILE * 8], u32, tag="idx_acc")
            idx_c = accp.tile([P, NTILE * TOPK], u32, tag="idx_c")
            load_ps = ldpool.tile([E, 1], f32, tag="load")

            # hoist the exp ACT-table load into the DMA-wait prologue
            warm = constp.tile([1, 1], f32, tag="warm")
            nc.scalar.activation(warm[:], bias_sb[0:1, 0:1], AF.Exp)

            def emit_mm(g):
                """DMAs + the 48 fp16 matmuls + bias-copy for group g."""
                xh_t = xgpool.tile([P, GW], f16, tag="xh")
                nparts = 2
                step = GW // nparts
                for q in range(nparts):
                    nc.sync.dma_start(
                        xh_t[:, q * step:(q + 1) * step],
                        xh_d[g * P:(g + 1) * P, q * step:(q + 1) * step])
                xl_t = xgpool.tile([P, GW], f16, tag="xl")
                hw_ = GW // 2
                nc.sync.dma_start(xl_t[:, 0:hw_],
                                  xl_d[g * P:(g + 1) * P, 0:hw_])
                nc.sync.dma_start(xl_t[:, hw_:GW],
                                  xl_d[g * P:(g + 1) * P, hw_:GW])

                # hi terms first (only need the xh plane)
                lg = lgpool.tile([E, GT], f32, tag="lg")
                for j in range(NCHUNK):
                    nc.tensor.matmul(lg[:], wh_sb[:, j * E:(j + 1) * E],
                                     xh_t[:, j * GT:(j + 1) * GT],
                                     start=(j == 0), stop=False)
                for j in range(NCHUNK):
                    nc.tensor.matmul(lg[:], wl_sb[:, j * E:(j + 1) * E],
                                     xh_t[:, j * GT:(j + 1) * GT],
                                     start=False, stop=False)
                for j in range(NCHUNK):
                    nc.tensor.matmul(lg[:], wh_sb[:, j * E:(j + 1) * E],
                                     xl_t[:, j * GT:(j + 1) * GT],
                                     start=False, stop=(j == NCHUNK - 1))

                # PSUM -> SBUF with bias add (per-partition = per-expert)
                lgs = lgsbp.tile([E, GT], f32, tag="lgs")
                nc.scalar.activation(lgs[:], lg[:], AF.Identity,
                                     bias=bias_sb[:, 0:1], scale=1.0)
                return lgs

            def emit_post(g, lgs):
                """Per-128-token-tile softmax/top-k for group g."""
                for k in range(NGROUP):
                    i = NGROUP * g + k
                    # transpose [32, 128] -> [128, 32] on the PE
                    lgT = ltpool.tile([P, E], f32, tag="lgT_ps")
                    nc.tensor.transpose(
                        lgT[:], lgs[0:E, 128 * k:128 * (k + 1)], ident[:])

                    # p~ = exp(true logits)
                    p_t = tp.tile([P, E], f32, tag="p")
                    nc.scalar.activation(p_t[:], lgT[:], AF.Exp,
                                         scale=EXP_SCALE)
                    # S = per-token softmax denom (DVE reduce)
                    s_t = smallp.tile([P, 1], f32, tag="s")
                    nc.vector.reduce_sum(s_t[:], p_t[:], axis=AX.X)
                    rs_t = smallp.tile([P, 1], f32, tag="rs")
                    nc.vector.reciprocal(rs_t[:], s_t[:])

                    # top-8 values (desc) + indices
                    mx8 = smallp.tile([P, 8], f32, tag="mx8")
                    nc.vector.max(mx8[:], p_t[:])
                    nc.vector.max_index(idx_acc[:, 8 * i:8 * i + 8],
                                        mx8[:], p_t[:])

                    t4 = smallp.tile([P, 1], f32, tag="t4")
                    nc.vector.reduce_sum(t4[:], mx8[:, 0:4], axis=AX.X)
                    r4 = smallp.tile([P, 1], f32, tag="r4")
                    nc.vector.reciprocal(r4[:], t4[:])

                    # mask = p~ * ((p~ >= v4) * r4)   (2 fused DVE ops)
                    ge = tp.tile([P, E], f32, tag="ge")
                    nc.vector.tensor_scalar(ge[:], p_t[:], mx8[:, 3:4],
                                            r4[:, 0:1], op0=Alu.is_ge,
                                            op1=Alu.mult)
                    nc.vector.tensor_mul(mask_acc[:, E * i:E * (i + 1)],
                                         p_t[:], ge[:])

                    # expert-load partial: load[e] += sum_t p~[t,e] / S[t]
                    nc.tensor.matmul(load_ps[:], p_t[:], rs_t[:],
                                     start=(i == 0), stop=(i == NTILE - 1))
                # compact this group's top-4 index columns (overlapped)
                nc.vector.tensor_copy(
                    idx_c[:, 4 * TOPK * g:4 * TOPK * (g + 1)]
                    .rearrange("p (k e) -> p k e", e=TOPK),
                    idx_acc[:, 32 * g:32 * (g + 1)]
                    .rearrange("p (k e) -> p k e", e=8)[:, :, 0:TOPK])
                # stream the mask out per finished group, overlapped
                if g < NGROUP - 1:
                    q = NTILE * E // NGROUP
                    nc.sync.dma_start(mask_d[:, g * q:(g + 1) * q],
                                      mask_acc[:, g * q:(g + 1) * q])

            # software-pipelined emission: postproc for group g is issued
            # after group g+1's matmuls so the in-order PE queue never
            # stalls on the (DVE/ACT) bias-copy of the current group
            prev = None
            for g in range(NGROUP):
                lgs = emit_mm(g)
                if prev is not None:
                    emit_post(prev[0], prev[1])
                prev = (g, lgs)
            emit_post(prev[0], prev[1])

            # ---- outputs ----
            q = NTILE * E // NGROUP
            nc.sync.dma_start(mask_d[:, (NGROUP - 1) * q:],
                              mask_acc[:, (NGROUP - 1) * q:])
            nc.sync.dma_start(idx_d[:], idx_c[:].bitcast(i32))
            ld_sb = smallp.tile([E, 1], f32, tag="ld_sb")
            nc.vector.tensor_copy(ld_sb[:], load_ps[:])
            nc.sync.dma_start(load_d[:], ld_sb[:])

    nc.finalize()
    return nc


def _get_nc():
    if "nc" not in _CACHE:
        _CACHE["nc"] = _build_nc()
    return _CACHE["nc"]


def _split16(a32):
    """fp16 hi/lo split: a32 ~= hi + lo with ~2^-22 relative error."""
    hi = a32.astype(np.float16)
    lo = (a32 - hi.astype(np.float32)).astype(np.float16)
    return hi, lo


def _prep_inputs(hidden_states, router_weight, expert_bias):
    x = np.asarray(hidden_states, np.float32).reshape(T, H)
    w = np.asarray(router_weight, np.float32)
    b = np.asarray(expert_bias, np.float32)

    # W' = 4096 * W^T, chunk-major fp16 hi/lo [128, NCHUNK*E]
    wt = (WSCALE * w.T).reshape(NCHUNK, P, E).transpose(1, 0, 2) \
        .reshape(P, NCHUNK * E)
    wh, wl = _split16(np.ascontiguousarray(wt))
    bias_col = np.ascontiguousarray((XSCALE * WSCALE) * b.reshape(E, 1))

    in_maps = []
    for c in range(NCORES):
        xs = XSCALE * x[c * TPC:(c + 1) * TPC].T        # [H, TPC] f32
        xh, xl = _split16(xs)
        # [H, TPC] -> (j 16, p 128, g 4, t 512) -> [g, p, j, t] flat
        def lay(a):
            return np.ascontiguousarray(
                a.reshape(NCHUNK, P, NGROUP, GT).transpose(2, 1, 0, 3)
            ).reshape(NGROUP * P, NCHUNK * GT)
        in_maps.append({
            "xh": lay(xh), "xl": lay(xl),
            "wh": wh, "wl": wl, "bias": bias_col,
        })
    return in_maps


def _postprocess(results):
    mask_shards = []
    idx_shards = []
    for c in range(NCORES):
        m = results[c]["mask_out"].reshape(P, NTILE, E)
        mask_shards.append(np.ascontiguousarray(m.transpose(1, 0, 2))
                           .reshape(TPC, E))
        ii = results[c]["idx_out"].reshape(P, NTILE, TOPK)
        idx_shards.append(np.ascontiguousarray(ii.transpose(1, 0, 2))
                          .reshape(TPC, TOPK))
    mask_full = np.concatenate(mask_shards, 0).reshape(B, S, E)
    idx_full = np.concatenate(idx_shards, 0).reshape(B, S, TOPK)

    # balance loss from per-core expert-load partials (tiny host reduction,
    # mirrors the reference fp32 arithmetic)
    load_sum = np.zeros((E,), np.float32)
    for c in range(NCORES):
        load_sum = load_sum + results[c]["load_out"].reshape(E).astype(
            np.float32)
    expert_load = load_sum / np.float32(T)
    tgt = np.float32(1.0 / E)
    balance = np.sum(tgt * (np.log(tgt) - np.log(expert_load)),
                     dtype=np.float32) / np.float32(E)
    loss = np.float32(balance * np.float32(1e-4))
    return (mask_full.astype(np.float32), loss,
            idx_full.astype(np.int32))


def run_on_device(in_maps, trace=False, **kw):
    from concourse.bass_utils import run_bass_kernel_spmd
    nc = _get_nc()
    return run_bass_kernel_spmd(nc, in_maps, list(range(NCORES)),
                                trace=trace, **kw)


def kernel(hidden_states, router_weight, expert_bias):
    in_maps = _prep_inputs(hidden_states, router_weight, expert_bias)
    res = run_on_device(in_maps)
    return _postprocess(res.results)


if __name__ == "__main__":
    rng = np.random.default_rng(0)
    hs = rng.standard_normal((B, S, H), dtype=np.float32)
    w = (0.02 * rng.standard_normal((E, H))).astype(np.float32)
    b = np.zeros((E,), np.float32)
    out = kernel(hs, w, b)
    print([getattr(o, "shape", o) for o in out])
